# revision 1
# baseline (speedup 1.0000x reference)
"""Trainium2 Bass kernel for ChunkedSurpriseGatedSSD.

Strategy (v5, 74.5us vs 85us baseline)
--------------------------------------
Host gate chain + global-decay re-chunking into 128-token super-chunks (all
decay factors folded on host into fp16 operands referenced to each
super-chunk's mid-point log-decay), device program tuned around measured
bottlenecks:

* All four operand tensors interleaved into ONE contiguous DRAM image
  [128, NSUP, PPC, 448]; each multi-super group loads with a single HWDGE
  DMA of up-to-14KB-contiguous per-partition lines (281 GB/s vs 259 for the
  v1 four-tensor layout). Splitting a group across both HWDGE rings was
  measured SLOWER (each ring drops to ~150 GB/s) - keep one ring for input.
* Causal mask runs fused with the mandatory PSUM drain on DVE (the fp32-PSUM
  1x tier is still the cheapest total; pre-draining via ScalarE or masking
  on gpsimd both measured slower). State update stays entirely on DVE
  (cross-engine state chains serialize the per-super loop): gt = dn*g0,
  then one fused scalar_tensor_tensor g1 = pp + gt straight from PSUM.
* Y drains via ScalarE to fp16 and ships per-group on the Activation HWDGE
  ring (halves write traffic vs fp32 and frees gpsimd entirely).
* Deep pipelining: 6 input buffers, 3 mask buffers, 3/3/2 PSUM banks.

Work is sharded over the 8 NeuronCores by (batch, head) pair: 32 pairs, 4 per
core; every core runs an identical program on different data (SPMD).
"""
import os
import sys

for _p in ("/opt/trn_rl_repo", "/root/.axon_site/_ro/trn_rl_repo"):
    if os.path.isdir(_p) and _p not in sys.path:
        sys.path.append(_p)

import numpy as np

CHUNK = 64
EMA_DECAY = 0.99
Bsz, S, H, P, N = 2, 4096, 16, 64, 128
CS = 128                 # device super-chunk (2 reference chunks)
NSUP = S // CS           # 32
NCORES = 8
PAIRS = Bsz * H          # 32
PPC = PAIRS // NCORES    # 4 pairs per core
LINE = P + N + CS + CS   # 448 fp16 per (partition, super, pair) input line
GROUPS = [(0, 2), (2, 2), (4, 4), (8, 4), (12, 4), (16, 4), (20, 4), (24, 4),
          (28, 4)]
OGS = 8                  # supers per output batch

_CACHE = {}


def host_gate_chain(X, A, Bm, log2_alpha_base, log2_beta, surprise_ema):
    """decay_scale sequence ds[nC] via err_c = mean(h_contrib_{c-1}^2)."""
    nC = S // CHUNK
    alpha_base = 1.0 - np.exp2(np.clip(log2_alpha_base, -3.32, -0.015))  # [H]
    beta = np.exp2(np.clip(log2_beta, -2.0, 2.0))                        # [H]

    A64 = A.astype(np.float64)
    ds = np.zeros(nC, np.float64)
    ema = surprise_ema.astype(np.float64).copy()
    err_next = None
    for c in range(nC):
        if c == 0:
            decay_scale = 1.0
        else:
            err = err_next
            ema = EMA_DECAY * ema + (1.0 - EMA_DECAY) * err.mean(axis=0)
            normalized = err / (ema[None, :] + 1e-6)
            boost = np.maximum(np.tanh(beta[None, :] * normalized), 0.0)
            alpha = np.clip(alpha_base[None, :] + (1.0 - alpha_base[None, :]) * boost,
                            0.01, 0.999)
            decay_scale = float(np.mean(1.0 - alpha))
        ds[c] = decay_scale

        sl = slice(c * CHUNK, (c + 1) * CHUNK)
        Acs = np.cumsum(A64[:, sl, :] * decay_scale, axis=1)        # [B,cs,H]
        dte = np.exp(Acs[:, -1:, :] - Acs).astype(np.float32)       # [B,cs,H]
        Xs = X[:, sl] * dte[..., None]                              # [B,cs,H,P]
        Bt = np.ascontiguousarray(Bm[:, sl].transpose(0, 2, 3, 1))  # [B,H,N,cs]
        Xt = np.ascontiguousarray(Xs.transpose(0, 2, 1, 3))         # [B,H,cs,P]
        contrib = Bt @ Xt                                           # [B,H,N,P]
        err_next = np.square(contrib, dtype=np.float64).mean(axis=(-2, -1))
    return ds


def build_nc():
    import concourse.bacc as bacc
    import concourse.tile as tile
    from concourse import mybir

    f32 = mybir.dt.float32
    f16 = mybir.dt.float16
    Act = mybir.ActivationFunctionType
    Alu = mybir.AluOpType

    nc = bacc.Bacc("TRN2", debug=False)
    Inp = nc.dram_tensor("Inp", [128, NSUP, PPC, LINE], f16,
                         kind="ExternalInput").ap()
    Vec = nc.dram_tensor("Vec", [N, PPC, NSUP], f32, kind="ExternalInput").ap()
    Tri = nc.dram_tensor("Tri", [CS, CS], f32, kind="ExternalInput").ap()
    Yp = nc.dram_tensor("Yp", [CS, NSUP, PPC, P], f16,
                        kind="ExternalOutput").ap()

    with tile.TileContext(nc) as tc:
        with (
            tc.tile_pool(name="const", bufs=1) as const_pool,
            tc.tile_pool(name="state", bufs=1) as state_pool,
            tc.tile_pool(name="tin", bufs=6) as in_pool,
            tc.tile_pool(name="mst", bufs=3) as mst_pool,
            tc.tile_pool(name="yout", bufs=2) as yout_pool,
            tc.tile_pool(name="pcb", bufs=3, space="PSUM") as pcb_pool,
            tc.tile_pool(name="py", bufs=3, space="PSUM") as py_pool,
            tc.tile_pool(name="pp", bufs=2, space="PSUM") as pp_pool,
        ):
            vecs = const_pool.tile([N, PPC, NSUP], f32)
            nc.sync.dma_start(out=vecs, in_=Vec)
            tri = const_pool.tile([CS, CS], f32)
            nc.sync.dma_start(out=tri, in_=Tri)

            # double-buffered state for all 4 pairs: h~ [N, pair, P]
            hst = []
            for k in range(3):
                t = state_pool.tile([N, PPC, P], f16, name=f"h_{k}", tag=f"h_{k}")
                nc.vector.memset(t, 0.0)
                hst.append(t)

            ysb = None
            for s0, gs in GROUPS:
                tin = in_pool.tile([128, 4, PPC, LINE], f16, name="tin",
                                   tag="tin")
                nc.sync.dma_start(out=tin[:, 0:gs], in_=Inp[:, s0:s0 + gs])
                ysb = yout_pool.tile([CS, 4, PPC, P], f16, name="ysb",
                                     tag="ysb")
                for off in range(gs):
                    Ssup = s0 + off
                    xin = tin[:, off, :, 0:P]
                    bin_ = tin[:, off, :, P:P + N]
                    btin = tin[:, off, :, P + N:P + N + CS]
                    ctin = tin[:, off, :, P + N + CS:LINE]

                    # mm1: CBt[j,i] (dfs folded via Ct') per pair into PSUM
                    pcb = pcb_pool.tile([CS, PPC, CS], f32, name="pcb",
                                        tag="pcb")
                    for p in range(PPC):
                        nc.tensor.matmul(pcb[:, p, :], btin[:, p, :],
                                         ctin[:, p, :], start=True, stop=True)
                    # state rescale first so the cross-super chain leads the
                    # DVE queue: gt = dn * g0
                    g0 = hst[Ssup % 2]
                    g1 = hst[(Ssup + 1) % 2]
                    gt = hst[2]
                    dnb = vecs[:, :, Ssup:Ssup + 1].broadcast_to([N, PPC, P])
                    nc.vector.tensor_mul(gt, g0, dnb)
                    # causal mask fused with the PSUM drain (DVE)
                    mst = mst_pool.tile([CS, PPC, CS], f16, name="mst",
                                        tag="mst")
                    tri_b = tri.unsqueeze(1).broadcast_to([CS, PPC, CS])
                    nc.vector.tensor_mul(mst, pcb, tri_b)

                    py = py_pool.tile([CS, PPC, P], f32, name="py", tag="py")
                    pp = pp_pool.tile([N, PPC, P], f32, name="pp", tag="pp")

                    for p in range(PPC):
                        nc.tensor.matmul(py[:, p, :], mst[:, p, :],
                                         xin[:, p, :], start=True,
                                         stop=(Ssup == 0))
                        if Ssup > 0:
                            nc.tensor.matmul(py[:, p, :], ctin[:, p, :],
                                             g0[:, p, :], start=False,
                                             stop=True)
                        nc.tensor.matmul(pp[:, p, :], bin_[:, p, :],
                                         xin[:, p, :], start=True, stop=True)
                    # state: g1 = gt + pp in one fused DVE op (pp carries
                    # the dn fold)
                    nc.vector.scalar_tensor_tensor(out=g1, in0=pp, scalar=1.0,
                                                   in1=gt, op0=Alu.mult,
                                                   op1=Alu.add)

                    # Y: PSUM -> fp16 SBUF (ScalarE); ship per group below
                    nc.scalar.activation(out=ysb[:, off], in_=py,
                                         func=Act.Copy)
                nc.scalar.dma_start(out=Yp[:, s0:s0 + gs], in_=ysb[:, 0:gs])

    nc.compile()
    return nc


def _pack_inputs(X, A, Bm, Cm, ds):
    """Interleaved fp16 input image + fp16 decay vectors (mid-referenced)."""
    w = np.repeat(ds, CHUNK)                                     # [S]
    Acsg = np.cumsum(A.astype(np.float64) * w[None, :, None], axis=1)  # [B,S,H]

    Ac = Acsg.reshape(Bsz, NSUP, CS, H)
    a_end = Ac[:, :, -1, :]                                      # [B,NSUP,H]
    a_start = np.zeros_like(a_end)
    a_start[:, 1:] = a_end[:, :-1]
    r = 0.5 * (a_start + a_end)                                  # [B,NSUP,H]
    acs = Ac - r[:, :, None, :]                                  # centered, f64
    idf = np.exp(-acs).astype(np.float32)                        # [B,NSUP,CS,H]
    dfs = np.exp(acs).astype(np.float32)
    dnext = np.ones((Bsz, NSUP, H))
    dnext[:, :-1] = np.exp(r[:, 1:] - r[:, :-1])
    dn_b = np.broadcast_to(dnext[:, :, None, :], idf.shape).astype(np.float32)

    def pack_tmaj(T, D):   # [B,S,H,D] -> [NSUP, CS, pair, D]
        return T.reshape(Bsz, NSUP, CS, H, D).transpose(1, 2, 0, 3, 4) \
                .reshape(NSUP, CS, PAIRS, D)

    def pack_nmaj(T, D):   # [B,S,H,D] -> [NSUP, D, pair, CS]
        return T.reshape(Bsz, NSUP, CS, H, D).transpose(1, 4, 0, 3, 2) \
                .reshape(NSUP, D, PAIRS, CS)

    f16 = np.float16
    Xa = pack_tmaj(X, P)
    # row-axis fold for B: idf[t] * delta_next  -> [NSUP, CS, pair, 1]
    idfd = (idf * dn_b).transpose(1, 2, 0, 3).reshape(NSUP, CS, PAIRS, 1)
    Ba = pack_tmaj(Bm, N) * idfd
    # free-axis folds: idf[j] for Bt, dfs[i] for Ct -> [NSUP, 1, pair, CS]
    idf_pair = idf.transpose(1, 0, 3, 2).reshape(NSUP, 1, PAIRS, CS)
    dfs_pair = dfs.transpose(1, 0, 3, 2).reshape(NSUP, 1, PAIRS, CS)
    Bta = pack_nmaj(Bm, N) * idf_pair
    Cta = pack_nmaj(Cm, N) * dfs_pair

    # interleave into [128, NSUP, PAIRS, LINE]
    Inq = np.concatenate([Xa.transpose(1, 0, 2, 3),
                          Ba.transpose(1, 0, 2, 3),
                          Bta.transpose(1, 0, 2, 3),
                          Cta.transpose(1, 0, 2, 3)], axis=-1).astype(f16)

    # dn per (pair, S), duplicated across partitions: [N, PAIRS, NSUP]
    dn = dnext.transpose(0, 2, 1).reshape(PAIRS, NSUP).astype(np.float32)
    vec = np.broadcast_to(dn[None, :, :], (N, PAIRS, NSUP))

    tri = (np.arange(CS)[None, :] >= np.arange(CS)[:, None]).astype(np.float32)

    in_maps = []
    for k in range(NCORES):
        sl = slice(k * PPC, (k + 1) * PPC)
        in_maps.append({
            "Inp": np.ascontiguousarray(Inq[:, :, sl, :]),
            "Vec": np.ascontiguousarray(vec[:, sl, :]),
            "Tri": tri,
        })
    return in_maps


def kernel(X, A, Bm, Cm, log2_alpha_base, log2_beta, surprise_ema):
    X = np.ascontiguousarray(np.asarray(X, np.float32))
    A = np.ascontiguousarray(np.asarray(A, np.float32))
    Bm = np.ascontiguousarray(np.asarray(Bm, np.float32))
    Cm = np.ascontiguousarray(np.asarray(Cm, np.float32))
    log2_alpha_base = np.asarray(log2_alpha_base, np.float32)
    log2_beta = np.asarray(log2_beta, np.float32)
    surprise_ema = np.asarray(surprise_ema, np.float32)

    ds = host_gate_chain(X, A, Bm, log2_alpha_base, log2_beta, surprise_ema)
    in_maps = _pack_inputs(X, A, Bm, Cm, ds)

    if "nc" not in _CACHE:
        _CACHE["nc"] = build_nc()
    nc = _CACHE["nc"]

    from concourse.bass_utils import run_bass_kernel_spmd
    res = run_bass_kernel_spmd(nc, in_maps, core_ids=list(range(NCORES)))

    # gather: Yp [CS, NSUP, PPC, P] per core -> Y [B, S, H, P]
    Y = np.empty((PAIRS, NSUP, CS, P), np.float32)
    for k in range(NCORES):
        yk = res.results[k]["Yp"]                   # [CS, NSUP, PPC, P]
        Y[k * PPC:(k + 1) * PPC] = yk.transpose(2, 1, 0, 3)
    Y = Y.reshape(Bsz, H, S, P).transpose(0, 2, 1, 3)
    return np.ascontiguousarray(Y)



# revision 2
# speedup vs baseline: 1.0127x; 1.0127x over previous
"""Trainium2 Bass kernel for ChunkedSurpriseGatedSSD.

Strategy (v6)
-------------
Host gate chain + global-decay re-chunking into 128-token super-chunks (decay
factors folded on host into fp16 operands referenced to each super-chunk's
mid-point log-decay).

v6 changes vs v5 (83/71us):
* Row-major B copy is no longer DMA'd: LINE 448 -> 320 fp16 (input image
  14.7MB -> 10.5MB).  B_row is reconstructed on-chip with a PE transpose
  (is_transpose matmul, fp16 PSUM output) and drained to SBUF by ScalarE in
  2-super batches.
* Engine rebalance: DVE keeps only the causal-mask drain and the state add
  (both PSUM-bound); the per-super state decay multiply moved to GpSimd
  (SBUF-only op); Y and B_row PSUM->SBUF drains live on ScalarE.
* State update restructured: pp' (no dn fold) + g0 -> fp32 t1 on DVE, then
  g1 = dn * t1 on GpSimd (fp32 t1 avoids fp16 overflow of the un-decayed sum).
* Input stream in 6 DMAs with 8-super groups (up to 2.6MB/transfer) on the
  SP HWDGE ring; consts + output on the Act ring.

Work is sharded over the 8 NeuronCores by (batch, head) pair: 32 pairs, 4 per
core; every core runs an identical program on different data (SPMD).
"""
import os
import sys

for _p in ("/opt/trn_rl_repo", "/root/.axon_site/_ro/trn_rl_repo"):
    if os.path.isdir(_p) and _p not in sys.path:
        sys.path.append(_p)

import numpy as np

CHUNK = 64
EMA_DECAY = 0.99
Bsz, S, H, P, N = 2, 4096, 16, 64, 128
CS = 128                 # device super-chunk (2 reference chunks)
NSUP = S // CS           # 32
NCORES = 8
PAIRS = Bsz * H          # 32
PPC = PAIRS // NCORES    # 4 pairs per core
LINE = P + N + N         # 320 fp16 per (partition, super, pair) input line
GROUPS = [(0, 2), (2, 2), (4, 4), (8, 8), (16, 8), (24, 8)]
GS_MAX = 8

_CACHE = {}


def host_gate_chain(X, A, Bm, log2_alpha_base, log2_beta, surprise_ema):
    """decay_scale sequence ds[nC] via err_c = mean(h_contrib_{c-1}^2)."""
    nC = S // CHUNK
    alpha_base = 1.0 - np.exp2(np.clip(log2_alpha_base, -3.32, -0.015))  # [H]
    beta = np.exp2(np.clip(log2_beta, -2.0, 2.0))                        # [H]

    A64 = A.astype(np.float64)
    ds = np.zeros(nC, np.float64)
    ema = surprise_ema.astype(np.float64).copy()
    err_next = None
    for c in range(nC):
        if c == 0:
            decay_scale = 1.0
        else:
            err = err_next
            ema = EMA_DECAY * ema + (1.0 - EMA_DECAY) * err.mean(axis=0)
            normalized = err / (ema[None, :] + 1e-6)
            boost = np.maximum(np.tanh(beta[None, :] * normalized), 0.0)
            alpha = np.clip(alpha_base[None, :] + (1.0 - alpha_base[None, :]) * boost,
                            0.01, 0.999)
            decay_scale = float(np.mean(1.0 - alpha))
        ds[c] = decay_scale

        sl = slice(c * CHUNK, (c + 1) * CHUNK)
        Acs = np.cumsum(A64[:, sl, :] * decay_scale, axis=1)        # [B,cs,H]
        dte = np.exp(Acs[:, -1:, :] - Acs).astype(np.float32)       # [B,cs,H]
        Xs = X[:, sl] * dte[..., None]                              # [B,cs,H,P]
        Bt = np.ascontiguousarray(Bm[:, sl].transpose(0, 2, 3, 1))  # [B,H,N,cs]
        Xt = np.ascontiguousarray(Xs.transpose(0, 2, 1, 3))         # [B,H,cs,P]
        contrib = Bt @ Xt                                           # [B,H,N,P]
        err_next = np.square(contrib, dtype=np.float64).mean(axis=(-2, -1))
    return ds


def build_nc():
    import concourse.bacc as bacc
    import concourse.tile as tile
    from concourse import mybir

    f32 = mybir.dt.float32
    f16 = mybir.dt.float16
    Act = mybir.ActivationFunctionType
    Alu = mybir.AluOpType

    nc = bacc.Bacc("TRN2", debug=False)
    Inp = nc.dram_tensor("Inp", [128, NSUP, PPC, LINE], f16,
                         kind="ExternalInput").ap()
    Vec = nc.dram_tensor("Vec", [N, PPC, NSUP], f32, kind="ExternalInput").ap()
    Tri = nc.dram_tensor("Tri", [CS, CS], f32, kind="ExternalInput").ap()
    Idn = nc.dram_tensor("Idn", [N, N], f16, kind="ExternalInput").ap()
    Yp = nc.dram_tensor("Yp", [CS, NSUP, PPC, P], f16,
                        kind="ExternalOutput").ap()

    with tile.TileContext(nc) as tc:
        with (
            tc.tile_pool(name="const", bufs=1) as const_pool,
            tc.tile_pool(name="state", bufs=1) as state_pool,
            tc.tile_pool(name="tin", bufs=3) as in_pool,
            tc.tile_pool(name="mst", bufs=3) as mst_pool,
            tc.tile_pool(name="brow", bufs=2) as brow_pool,
            tc.tile_pool(name="yout", bufs=2) as yout_pool,
            tc.tile_pool(name="pcb", bufs=2, space="PSUM") as pcb_pool,
            tc.tile_pool(name="pt", bufs=2, space="PSUM") as pt_pool,
            tc.tile_pool(name="py", bufs=2, space="PSUM") as py_pool,
            tc.tile_pool(name="pp", bufs=2, space="PSUM") as pp_pool,
        ):
            vecs = const_pool.tile([N, PPC, NSUP], f32)
            nc.scalar.dma_start(out=vecs, in_=Vec)
            tri = const_pool.tile([CS, CS], f32)
            nc.scalar.dma_start(out=tri, in_=Tri)
            ident = const_pool.tile([N, N], f16)
            nc.scalar.dma_start(out=ident, in_=Idn)

            # state: g ping-pong (f16) + t1 ping-pong (f32, pre-decay sum)
            gst = []
            for k in range(2):
                t = state_pool.tile([N, PPC, P], f16, name=f"g_{k}",
                                    tag=f"g_{k}")
                nc.vector.memset(t, 0.0)
                gst.append(t)
            t1st = [state_pool.tile([N, PPC, P], f32, name=f"t1_{k}",
                                    tag=f"t1_{k}") for k in range(2)]

            ysb = None
            for g0s, gs in GROUPS:
                tin = in_pool.tile([128, GS_MAX, PPC, LINE], f16, name="tin",
                                   tag="tin")
                nc.sync.dma_start(out=tin[:, 0:gs], in_=Inp[:, g0s:g0s + gs])
                for b in range(gs // 2):
                    s0 = g0s + 2 * b
                    if s0 % 4 == 0:
                        ysb = yout_pool.tile([CS, 4, PPC, P], f16, name="ysb",
                                             tag="ysb")
                    # --- per-2-super block ---
                    pt = pt_pool.tile([CS, 2, PPC, N], f16, name="pt",
                                      tag="pt")
                    pcbs = []
                    for so in (0, 1):
                        s = s0 + so
                        off = s - g0s
                        btin = tin[:, off, :, P:P + N]
                        ctin = tin[:, off, :, P + N:LINE]
                        pcb = pcb_pool.tile([CS, PPC, CS], f32, name="pcb",
                                            tag="pcb")
                        for p in range(PPC):
                            nc.tensor.matmul(pt[:, so, p, :], btin[:, p, :],
                                             ident, is_transpose=True)
                            nc.tensor.matmul(pcb[:, p, :], btin[:, p, :],
                                             ctin[:, p, :], start=True,
                                             stop=True)
                        # causal mask fused with the PSUM drain (DVE)
                        mst = mst_pool.tile([CS, PPC, CS], f16, name="mst",
                                            tag="mst")
                        tri_b = tri.unsqueeze(1).broadcast_to([CS, PPC, CS])
                        nc.vector.tensor_mul(mst, pcb, tri_b)
                        pcbs.append(mst)
                    # B_row drain for both supers in one ScalarE op
                    brow = brow_pool.tile([CS, 2, PPC, N], f16, name="brow",
                                          tag="brow")
                    nc.scalar.activation(out=brow, in_=pt, func=Act.Copy)

                    py = py_pool.tile([CS, 2, PPC, P], f32, name="py",
                                      tag="py")
                    for so in (0, 1):
                        s = s0 + so
                        off = s - g0s
                        xin = tin[:, off, :, 0:P]
                        ctin = tin[:, off, :, P + N:LINE]
                        mst = pcbs[so]
                        g0 = gst[s % 2]
                        g1 = gst[(s + 1) % 2]
                        t1 = t1st[s % 2]
                        pp = pp_pool.tile([N, PPC, P], f32, name="pp",
                                          tag="pp")
                        for p in range(PPC):
                            nc.tensor.matmul(py[:, so, p, :], mst[:, p, :],
                                             xin[:, p, :], start=True,
                                             stop=(s == 0))
                            if s > 0:
                                nc.tensor.matmul(py[:, so, p, :],
                                                 ctin[:, p, :], g0[:, p, :],
                                                 start=False, stop=True)
                            nc.tensor.matmul(pp[:, p, :],
                                             brow[:, so, p, :],
                                             xin[:, p, :], start=True,
                                             stop=True)
                        # state: t1 = pp + g0 (DVE, f32)  ;  g1 = dn*t1 (GpSimd)
                        nc.vector.scalar_tensor_tensor(out=t1, in0=pp,
                                                       scalar=1.0, in1=g0,
                                                       op0=Alu.mult,
                                                       op1=Alu.add)
                        dnb = vecs[:, :, s:s + 1].broadcast_to([N, PPC, P])
                        nc.gpsimd.tensor_mul(g1, t1, dnb)
                    # Y drain for both supers in one ScalarE op
                    nc.scalar.activation(out=ysb[:, (s0 % 4):(s0 % 4) + 2],
                                         in_=py, func=Act.Copy)
                    if s0 % 4 == 2:
                        nc.scalar.dma_start(out=Yp[:, s0 - 2:s0 + 2],
                                            in_=ysb)

    nc.compile()
    return nc


def _pack_inputs(X, A, Bm, Cm, ds):
    """Interleaved fp16 input image + decay vectors (mid-referenced)."""
    w = np.repeat(ds, CHUNK)                                     # [S]
    Acsg = np.cumsum(A.astype(np.float64) * w[None, :, None], axis=1)  # [B,S,H]

    Ac = Acsg.reshape(Bsz, NSUP, CS, H)
    a_end = Ac[:, :, -1, :]                                      # [B,NSUP,H]
    a_start = np.zeros_like(a_end)
    a_start[:, 1:] = a_end[:, :-1]
    r = 0.5 * (a_start + a_end)                                  # [B,NSUP,H]
    acs = Ac - r[:, :, None, :]                                  # centered, f64
    idf = np.exp(-acs).astype(np.float32)                        # [B,NSUP,CS,H]
    dfs = np.exp(acs).astype(np.float32)
    dnext = np.ones((Bsz, NSUP, H))
    dnext[:, :-1] = np.exp(r[:, 1:] - r[:, :-1])

    def pack_tmaj(T, D):   # [B,S,H,D] -> [NSUP, CS, pair, D]
        return T.reshape(Bsz, NSUP, CS, H, D).transpose(1, 2, 0, 3, 4) \
                .reshape(NSUP, CS, PAIRS, D)

    def pack_nmaj(T, D):   # [B,S,H,D] -> [NSUP, D, pair, CS]
        return T.reshape(Bsz, NSUP, CS, H, D).transpose(1, 4, 0, 3, 2) \
                .reshape(NSUP, D, PAIRS, CS)

    f16 = np.float16
    Xa = pack_tmaj(X, P)
    # free-axis folds: idf[j] for Bt, dfs[i] for Ct -> [NSUP, 1, pair, CS]
    idf_pair = idf.transpose(1, 0, 3, 2).reshape(NSUP, 1, PAIRS, CS)
    dfs_pair = dfs.transpose(1, 0, 3, 2).reshape(NSUP, 1, PAIRS, CS)
    Bta = pack_nmaj(Bm, N) * idf_pair
    Cta = pack_nmaj(Cm, N) * dfs_pair

    # interleave into [128, NSUP, PAIRS, LINE]
    Inq = np.concatenate([Xa.transpose(1, 0, 2, 3),
                          Bta.transpose(1, 0, 2, 3),
                          Cta.transpose(1, 0, 2, 3)], axis=-1).astype(f16)

    # dn per (pair, S), duplicated across partitions: [N, PAIRS, NSUP]
    dn = dnext.transpose(0, 2, 1).reshape(PAIRS, NSUP).astype(np.float32)
    vec = np.broadcast_to(dn[None, :, :], (N, PAIRS, NSUP))

    tri = (np.arange(CS)[None, :] >= np.arange(CS)[:, None]).astype(np.float32)
    idn = np.eye(N, dtype=f16)

    in_maps = []
    for k in range(NCORES):
        sl = slice(k * PPC, (k + 1) * PPC)
        in_maps.append({
            "Inp": np.ascontiguousarray(Inq[:, :, sl, :]),
            "Vec": np.ascontiguousarray(vec[:, sl, :]),
            "Tri": tri,
            "Idn": idn,
        })
    return in_maps


def kernel(X, A, Bm, Cm, log2_alpha_base, log2_beta, surprise_ema):
    X = np.ascontiguousarray(np.asarray(X, np.float32))
    A = np.ascontiguousarray(np.asarray(A, np.float32))
    Bm = np.ascontiguousarray(np.asarray(Bm, np.float32))
    Cm = np.ascontiguousarray(np.asarray(Cm, np.float32))
    log2_alpha_base = np.asarray(log2_alpha_base, np.float32)
    log2_beta = np.asarray(log2_beta, np.float32)
    surprise_ema = np.asarray(surprise_ema, np.float32)

    ds = host_gate_chain(X, A, Bm, log2_alpha_base, log2_beta, surprise_ema)
    in_maps = _pack_inputs(X, A, Bm, Cm, ds)

    if "nc" not in _CACHE:
        _CACHE["nc"] = build_nc()
    nc = _CACHE["nc"]

    from concourse.bass_utils import run_bass_kernel_spmd
    res = run_bass_kernel_spmd(nc, in_maps, core_ids=list(range(NCORES)))

    # gather: Yp [CS, NSUP, PPC, P] per core -> Y [B, S, H, P]
    Y = np.empty((PAIRS, NSUP, CS, P), np.float32)
    for k in range(NCORES):
        yk = res.results[k]["Yp"]                   # [CS, NSUP, PPC, P]
        Y[k * PPC:(k + 1) * PPC] = yk.transpose(2, 1, 0, 3)
    Y = Y.reshape(Bsz, H, S, P).transpose(0, 2, 1, 3)
    return np.ascontiguousarray(Y)


# revision 5
# speedup vs baseline: 1.0581x; 1.0448x over previous
"""Trainium2 Bass kernel for ChunkedSurpriseGatedSSD.

Strategy (v6)
-------------
Host gate chain + global-decay re-chunking into 128-token super-chunks (decay
factors folded on host into fp16 operands referenced to each super-chunk's
mid-point log-decay).

v6 changes vs v5 (83/71us):
* Row-major B copy is no longer DMA'd: LINE 448 -> 320 fp16 (input image
  14.7MB -> 10.5MB).  B_row is reconstructed on-chip with a PE transpose
  (is_transpose matmul, fp16 PSUM output) and drained to SBUF by ScalarE in
  2-super batches.
* Engine rebalance: DVE keeps only the causal-mask drain and the state add
  (both PSUM-bound); the per-super state decay multiply moved to GpSimd
  (SBUF-only op); Y and B_row PSUM->SBUF drains live on ScalarE.
* State update restructured: pp' (no dn fold) + g0 -> fp32 t1 on DVE, then
  g1 = dn * t1 on GpSimd (fp32 t1 avoids fp16 overflow of the un-decayed sum).
* Input stream in 6 DMAs with 8-super groups (up to 2.6MB/transfer) on the
  SP HWDGE ring; consts + output on the Act ring.

Work is sharded over the 8 NeuronCores by (batch, head) pair: 32 pairs, 4 per
core; every core runs an identical program on different data (SPMD).
"""
import os
import sys

for _p in ("/opt/trn_rl_repo", "/root/.axon_site/_ro/trn_rl_repo"):
    if os.path.isdir(_p) and _p not in sys.path:
        sys.path.append(_p)

import numpy as np

CHUNK = 64
EMA_DECAY = 0.99
Bsz, S, H, P, N = 2, 4096, 16, 64, 128
CS = 128                 # device super-chunk (2 reference chunks)
NSUP = S // CS           # 32
NCORES = 8
PAIRS = Bsz * H          # 32
PPC = PAIRS // NCORES    # 4 pairs per core
LINE = P + N + N         # 320 fp16 per (partition, super, pair) input line
GROUPS = [(0, 2), (2, 2), (4, 4), (8, 8), (16, 8), (24, 8)]
GS_MAX = 8

_CACHE = {}


def host_gate_chain(X, A, Bm, log2_alpha_base, log2_beta, surprise_ema):
    """decay_scale sequence ds[nC] via err_c = mean(h_contrib_{c-1}^2)."""
    nC = S // CHUNK
    alpha_base = 1.0 - np.exp2(np.clip(log2_alpha_base, -3.32, -0.015))  # [H]
    beta = np.exp2(np.clip(log2_beta, -2.0, 2.0))                        # [H]

    A64 = A.astype(np.float64)
    ds = np.zeros(nC, np.float64)
    ema = surprise_ema.astype(np.float64).copy()
    err_next = None
    for c in range(nC):
        if c == 0:
            decay_scale = 1.0
        else:
            err = err_next
            ema = EMA_DECAY * ema + (1.0 - EMA_DECAY) * err.mean(axis=0)
            normalized = err / (ema[None, :] + 1e-6)
            boost = np.maximum(np.tanh(beta[None, :] * normalized), 0.0)
            alpha = np.clip(alpha_base[None, :] + (1.0 - alpha_base[None, :]) * boost,
                            0.01, 0.999)
            decay_scale = float(np.mean(1.0 - alpha))
        ds[c] = decay_scale

        sl = slice(c * CHUNK, (c + 1) * CHUNK)
        Acs = np.cumsum(A64[:, sl, :] * decay_scale, axis=1)        # [B,cs,H]
        dte = np.exp(Acs[:, -1:, :] - Acs).astype(np.float32)       # [B,cs,H]
        Xs = X[:, sl] * dte[..., None]                              # [B,cs,H,P]
        Bt = np.ascontiguousarray(Bm[:, sl].transpose(0, 2, 3, 1))  # [B,H,N,cs]
        Xt = np.ascontiguousarray(Xs.transpose(0, 2, 1, 3))         # [B,H,cs,P]
        contrib = Bt @ Xt                                           # [B,H,N,P]
        err_next = np.square(contrib, dtype=np.float64).mean(axis=(-2, -1))
    return ds


def build_nc():
    import concourse.bacc as bacc
    import concourse.tile as tile
    from concourse import mybir

    f32 = mybir.dt.float32
    f16 = mybir.dt.float16
    Act = mybir.ActivationFunctionType
    Alu = mybir.AluOpType

    nc = bacc.Bacc("TRN2", debug=False)
    Inp = nc.dram_tensor("Inp", [128, NSUP, PPC, LINE], f16,
                         kind="ExternalInput").ap()
    Vec = nc.dram_tensor("Vec", [N, PPC, NSUP], f32, kind="ExternalInput").ap()
    Tri = nc.dram_tensor("Tri", [CS, CS], f32, kind="ExternalInput").ap()
    Idn = nc.dram_tensor("Idn", [N, N], f16, kind="ExternalInput").ap()
    Yp = nc.dram_tensor("Yp", [CS, NSUP, PPC, P], f16,
                        kind="ExternalOutput").ap()

    with tile.TileContext(nc) as tc:
        with (
            tc.tile_pool(name="const", bufs=1) as const_pool,
            tc.tile_pool(name="state", bufs=1) as state_pool,
            tc.tile_pool(name="tin", bufs=3) as in_pool,
            tc.tile_pool(name="mst", bufs=3) as mst_pool,
            tc.tile_pool(name="brow", bufs=2) as brow_pool,
            tc.tile_pool(name="yout", bufs=2) as yout_pool,
            tc.tile_pool(name="pcb", bufs=2, space="PSUM") as pcb_pool,
            tc.tile_pool(name="pt", bufs=2, space="PSUM") as pt_pool,
            tc.tile_pool(name="py", bufs=2, space="PSUM") as py_pool,
            tc.tile_pool(name="pp", bufs=2, space="PSUM") as pp_pool,
        ):
            vecs = const_pool.tile([N, PPC, NSUP], f32)
            nc.scalar.dma_start(out=vecs, in_=Vec)
            tri = const_pool.tile([CS, CS], f32)
            nc.scalar.dma_start(out=tri, in_=Tri)
            ident = const_pool.tile([N, N], f16)
            nc.scalar.dma_start(out=ident, in_=Idn)

            # state: g ping-pong (f16) + t1 ping-pong (f32, pre-decay sum)
            gst = []
            for k in range(2):
                t = state_pool.tile([N, PPC, P], f16, name=f"g_{k}",
                                    tag=f"g_{k}")
                nc.vector.memset(t, 0.0)
                gst.append(t)
            t1st = [state_pool.tile([N, PPC, P], f32, name=f"t1_{k}",
                                    tag=f"t1_{k}") for k in range(2)]

            # software pipeline over supers ("ticks"): tick s runs the
            # state-independent front of super s (T/pcb/mask, brow on odd s)
            # and the delayed mid+tail of super s-1 (intra/pp, then
            # inter/stt/gmul which depend on the state recurrence).  The
            # one-super delay keeps the stt->gmul->stt recurrence off the
            # PE/DVE critical path.
            grp_of = {}
            for gi, (g0s, gs) in enumerate(GROUPS):
                for s in range(g0s, g0s + gs):
                    grp_of[s] = (gi, g0s, gs)
            tins = {}
            pts = {}
            msts = {}
            brows = {}
            ppts = {}
            pys = {}
            ysbs = {}
            tri_b = tri.unsqueeze(1).broadcast_to([CS, PPC, CS])

            def xin_of(s):
                gi, g0s, _ = grp_of[s]
                return tins[gi][:, s - g0s, :, 0:P]

            def ctin_of(s):
                gi, g0s, _ = grp_of[s]
                return tins[gi][:, s - g0s, :, P + N:LINE]

            for s in range(NSUP + 1):
                if s < NSUP:
                    gi, g0s, gs = grp_of[s]
                    if s == g0s:
                        tin = in_pool.tile([128, GS_MAX, PPC, LINE], f16,
                                           name="tin", tag="tin")
                        nc.sync.dma_start(out=tin[:, 0:gs],
                                          in_=Inp[:, g0s:g0s + gs])
                        tins[gi] = tin
                    # --- front(s): T + pcb + mask ---
                    btin = tins[gi][:, s - g0s, :, P:P + N]
                    ctin = ctin_of(s)
                    if s % 2 == 0:
                        pts[s // 2] = pt_pool.tile([CS, 2, PPC, N], f16,
                                                   name="pt", tag="pt")
                    pt = pts[s // 2]
                    for p in range(PPC):
                        nc.tensor.matmul(pt[:, s % 2, p, :], btin[:, p, :],
                                         ident, is_transpose=True)
                    pcb = pcb_pool.tile([CS, PPC, CS], f32, name="pcb",
                                        tag="pcb")
                    for p in range(PPC):
                        nc.tensor.matmul(pcb[:, p, :], btin[:, p, :],
                                         ctin[:, p, :], start=True, stop=True)
                    mst = mst_pool.tile([CS, PPC, CS], f16, name="mst",
                                        tag="mst")
                    nc.vector.tensor_mul(mst, pcb, tri_b)
                    msts[s] = mst
                    if s % 2 == 1:
                        brow = brow_pool.tile([CS, 2, PPC, N], f16,
                                              name="brow", tag="brow")
                        nc.scalar.activation(out=brow, in_=pts[s // 2],
                                             func=Act.Copy)
                        brows[s // 2] = brow

                if s >= 1:
                    # --- mid(s-1): intra + pp ---
                    d = s - 1
                    b = d // 2
                    xin = xin_of(d)
                    if d % 2 == 0:
                        pys[b] = py_pool.tile([CS, 2, PPC, P], f32,
                                              name="py", tag="py")
                    py = pys[b]
                    mst = msts.pop(d)
                    ppt = pp_pool.tile([N, PPC, P], f32, name="pp", tag="pp")
                    g0 = gst[d % 2]
                    g1 = gst[(d + 1) % 2]
                    ctin = ctin_of(d)
                    for p in range(PPC):
                        nc.tensor.matmul(py[:, d % 2, p, :], mst[:, p, :],
                                         xin[:, p, :], start=True,
                                         stop=(d == 0))
                        if d > 0:
                            nc.tensor.matmul(py[:, d % 2, p, :],
                                             ctin[:, p, :], g0[:, p, :],
                                             start=False, stop=True)
                        nc.tensor.matmul(ppt[:, p, :],
                                         brows[b][:, d % 2, p, :],
                                         xin[:, p, :], start=True, stop=True)

                    # --- tail(s-1): state advance ---
                    t1 = t1st[d % 2]
                    nc.vector.scalar_tensor_tensor(out=t1, in0=ppt,
                                                   scalar=1.0, in1=g0,
                                                   op0=Alu.mult, op1=Alu.add)
                    dnb = vecs[:, :, d:d + 1].broadcast_to([N, PPC, P])
                    nc.gpsimd.tensor_mul(g1, t1, dnb)

                    if d % 2 == 1:
                        # Y drain for the finished 2-super pair on ScalarE
                        w = b // 4
                        if b % 4 == 0:
                            ysbs[w] = yout_pool.tile([CS, 8, PPC, P], f16,
                                                     name="ysb", tag="ysb")
                        off = 2 * (b % 4)
                        nc.scalar.activation(out=ysbs[w][:, off:off + 2],
                                             in_=pys.pop(b), func=Act.Copy)
                        if b % 4 == 3:
                            nc.scalar.dma_start(out=Yp[:, 8 * w:8 * w + 8],
                                                in_=ysbs.pop(w))

    nc.compile()
    return nc


def _pack_inputs(X, A, Bm, Cm, ds):
    """Interleaved fp16 input image + decay vectors (mid-referenced)."""
    w = np.repeat(ds, CHUNK)                                     # [S]
    Acsg = np.cumsum(A.astype(np.float64) * w[None, :, None], axis=1)  # [B,S,H]

    Ac = Acsg.reshape(Bsz, NSUP, CS, H)
    a_end = Ac[:, :, -1, :]                                      # [B,NSUP,H]
    a_start = np.zeros_like(a_end)
    a_start[:, 1:] = a_end[:, :-1]
    r = 0.5 * (a_start + a_end)                                  # [B,NSUP,H]
    acs = Ac - r[:, :, None, :]                                  # centered, f64
    idf = np.exp(-acs).astype(np.float32)                        # [B,NSUP,CS,H]
    dfs = np.exp(acs).astype(np.float32)
    dnext = np.ones((Bsz, NSUP, H))
    dnext[:, :-1] = np.exp(r[:, 1:] - r[:, :-1])

    def pack_tmaj(T, D):   # [B,S,H,D] -> [NSUP, CS, pair, D]
        return T.reshape(Bsz, NSUP, CS, H, D).transpose(1, 2, 0, 3, 4) \
                .reshape(NSUP, CS, PAIRS, D)

    def pack_nmaj(T, D):   # [B,S,H,D] -> [NSUP, D, pair, CS]
        return T.reshape(Bsz, NSUP, CS, H, D).transpose(1, 4, 0, 3, 2) \
                .reshape(NSUP, D, PAIRS, CS)

    f16 = np.float16
    Xa = pack_tmaj(X, P)
    # free-axis folds: idf[j] for Bt, dfs[i] for Ct -> [NSUP, 1, pair, CS]
    idf_pair = idf.transpose(1, 0, 3, 2).reshape(NSUP, 1, PAIRS, CS)
    dfs_pair = dfs.transpose(1, 0, 3, 2).reshape(NSUP, 1, PAIRS, CS)
    Bta = pack_nmaj(Bm, N) * idf_pair
    Cta = pack_nmaj(Cm, N) * dfs_pair

    # interleave into [128, NSUP, PAIRS, LINE]
    Inq = np.concatenate([Xa.transpose(1, 0, 2, 3),
                          Bta.transpose(1, 0, 2, 3),
                          Cta.transpose(1, 0, 2, 3)], axis=-1).astype(f16)

    # dn per (pair, S), duplicated across partitions: [N, PAIRS, NSUP]
    dn = dnext.transpose(0, 2, 1).reshape(PAIRS, NSUP).astype(np.float32)
    vec = np.broadcast_to(dn[None, :, :], (N, PAIRS, NSUP))

    tri = (np.arange(CS)[None, :] >= np.arange(CS)[:, None]).astype(np.float32)
    idn = np.eye(N, dtype=f16)

    in_maps = []
    for k in range(NCORES):
        sl = slice(k * PPC, (k + 1) * PPC)
        in_maps.append({
            "Inp": np.ascontiguousarray(Inq[:, :, sl, :]),
            "Vec": np.ascontiguousarray(vec[:, sl, :]),
            "Tri": tri,
            "Idn": idn,
        })
    return in_maps


def kernel(X, A, Bm, Cm, log2_alpha_base, log2_beta, surprise_ema):
    X = np.ascontiguousarray(np.asarray(X, np.float32))
    A = np.ascontiguousarray(np.asarray(A, np.float32))
    Bm = np.ascontiguousarray(np.asarray(Bm, np.float32))
    Cm = np.ascontiguousarray(np.asarray(Cm, np.float32))
    log2_alpha_base = np.asarray(log2_alpha_base, np.float32)
    log2_beta = np.asarray(log2_beta, np.float32)
    surprise_ema = np.asarray(surprise_ema, np.float32)

    ds = host_gate_chain(X, A, Bm, log2_alpha_base, log2_beta, surprise_ema)
    in_maps = _pack_inputs(X, A, Bm, Cm, ds)

    if "nc" not in _CACHE:
        _CACHE["nc"] = build_nc()
    nc = _CACHE["nc"]

    from concourse.bass_utils import run_bass_kernel_spmd
    res = run_bass_kernel_spmd(nc, in_maps, core_ids=list(range(NCORES)))

    # gather: Yp [CS, NSUP, PPC, P] per core -> Y [B, S, H, P]
    Y = np.empty((PAIRS, NSUP, CS, P), np.float32)
    for k in range(NCORES):
        yk = res.results[k]["Yp"]                   # [CS, NSUP, PPC, P]
        Y[k * PPC:(k + 1) * PPC] = yk.transpose(2, 1, 0, 3)
    Y = Y.reshape(Bsz, H, S, P).transpose(0, 2, 1, 3)
    return np.ascontiguousarray(Y)


# revision 6
# speedup vs baseline: 1.1045x; 1.0438x over previous
"""Trainium2 Bass kernel for ChunkedSurpriseGatedSSD.

Strategy (v6)
-------------
Host gate chain + global-decay re-chunking into 128-token super-chunks (decay
factors folded on host into fp16 operands referenced to each super-chunk's
mid-point log-decay).

v6 changes vs v5 (83/71us):
* Row-major B copy is no longer DMA'd: LINE 448 -> 320 fp16 (input image
  14.7MB -> 10.5MB).  B_row is reconstructed on-chip with a PE transpose
  (is_transpose matmul, fp16 PSUM output) and drained to SBUF by ScalarE in
  2-super batches.
* Engine rebalance: DVE keeps only the causal-mask drain and the state add
  (both PSUM-bound); the per-super state decay multiply moved to GpSimd
  (SBUF-only op); Y and B_row PSUM->SBUF drains live on ScalarE.
* State update restructured: pp' (no dn fold) + g0 -> fp32 t1 on DVE, then
  g1 = dn * t1 on GpSimd (fp32 t1 avoids fp16 overflow of the un-decayed sum).
* Input stream in 6 DMAs with 8-super groups (up to 2.6MB/transfer) on the
  SP HWDGE ring; consts + output on the Act ring.

Work is sharded over the 8 NeuronCores by (batch, head) pair: 32 pairs, 4 per
core; every core runs an identical program on different data (SPMD).
"""
import os
import sys

for _p in ("/opt/trn_rl_repo", "/root/.axon_site/_ro/trn_rl_repo"):
    if os.path.isdir(_p) and _p not in sys.path:
        sys.path.append(_p)

import numpy as np

CHUNK = 64
EMA_DECAY = 0.99
Bsz, S, H, P, N = 2, 4096, 16, 64, 128
CS = 128                 # device super-chunk (2 reference chunks)
NSUP = S // CS           # 32
NCORES = 8
PAIRS = Bsz * H          # 32
PPC = PAIRS // NCORES    # 4 pairs per core
LINE = P + N + N         # 320 fp16 per (partition, super, pair) input line
GROUPS = [(0, 2), (2, 2), (4, 4), (8, 4), (12, 4), (16, 4), (20, 4),
          (24, 4), (28, 4)]
GS_MAX = 4

_CACHE = {}


def host_gate_chain(X, A, Bm, log2_alpha_base, log2_beta, surprise_ema):
    """decay_scale sequence ds[nC] via err_c = mean(h_contrib_{c-1}^2)."""
    nC = S // CHUNK
    alpha_base = 1.0 - np.exp2(np.clip(log2_alpha_base, -3.32, -0.015))  # [H]
    beta = np.exp2(np.clip(log2_beta, -2.0, 2.0))                        # [H]

    A64 = A.astype(np.float64)
    ds = np.zeros(nC, np.float64)
    ema = surprise_ema.astype(np.float64).copy()
    err_next = None
    for c in range(nC):
        if c == 0:
            decay_scale = 1.0
        else:
            err = err_next
            ema = EMA_DECAY * ema + (1.0 - EMA_DECAY) * err.mean(axis=0)
            normalized = err / (ema[None, :] + 1e-6)
            boost = np.maximum(np.tanh(beta[None, :] * normalized), 0.0)
            alpha = np.clip(alpha_base[None, :] + (1.0 - alpha_base[None, :]) * boost,
                            0.01, 0.999)
            decay_scale = float(np.mean(1.0 - alpha))
        ds[c] = decay_scale

        sl = slice(c * CHUNK, (c + 1) * CHUNK)
        Acs = np.cumsum(A64[:, sl, :] * decay_scale, axis=1)        # [B,cs,H]
        dte = np.exp(Acs[:, -1:, :] - Acs).astype(np.float32)       # [B,cs,H]
        Xs = X[:, sl] * dte[..., None]                              # [B,cs,H,P]
        Bt = np.ascontiguousarray(Bm[:, sl].transpose(0, 2, 3, 1))  # [B,H,N,cs]
        Xt = np.ascontiguousarray(Xs.transpose(0, 2, 1, 3))         # [B,H,cs,P]
        contrib = Bt @ Xt                                           # [B,H,N,P]
        err_next = np.square(contrib, dtype=np.float64).mean(axis=(-2, -1))
    return ds


def build_nc():
    import concourse.bacc as bacc
    import concourse.tile as tile
    from concourse import mybir

    f32 = mybir.dt.float32
    f16 = mybir.dt.float16
    Act = mybir.ActivationFunctionType
    Alu = mybir.AluOpType

    nc = bacc.Bacc("TRN2", debug=False)
    Inp = nc.dram_tensor("Inp", [128, NSUP, PPC, LINE], f16,
                         kind="ExternalInput").ap()
    Vec = nc.dram_tensor("Vec", [N, PPC, NSUP], f32, kind="ExternalInput").ap()
    Tri = nc.dram_tensor("Tri", [CS, CS], f32, kind="ExternalInput").ap()
    Idn = nc.dram_tensor("Idn", [N, N], f16, kind="ExternalInput").ap()
    Yp = nc.dram_tensor("Yp", [CS, NSUP, PPC, P], f16,
                        kind="ExternalOutput").ap()

    with tile.TileContext(nc) as tc:
        with (
            tc.tile_pool(name="const", bufs=1) as const_pool,
            tc.tile_pool(name="state", bufs=1) as state_pool,
            tc.tile_pool(name="tin", bufs=6) as in_pool,
            tc.tile_pool(name="mst", bufs=3) as mst_pool,
            tc.tile_pool(name="brow", bufs=2) as brow_pool,
            tc.tile_pool(name="yout", bufs=2) as yout_pool,
            tc.tile_pool(name="pcb", bufs=2, space="PSUM") as pcb_pool,
            tc.tile_pool(name="pt", bufs=2, space="PSUM") as pt_pool,
            tc.tile_pool(name="py", bufs=2, space="PSUM") as py_pool,
            tc.tile_pool(name="pp", bufs=2, space="PSUM") as pp_pool,
        ):
            vecs = const_pool.tile([N, PPC, NSUP], f32)
            nc.scalar.dma_start(out=vecs, in_=Vec)
            tri = const_pool.tile([CS, CS], f32)
            nc.scalar.dma_start(out=tri, in_=Tri)
            ident = const_pool.tile([N, N], f16)
            nc.scalar.dma_start(out=ident, in_=Idn)

            # state: g ping-pong (f16) + t1 ping-pong (f32, pre-decay sum)
            gst = []
            for k in range(2):
                t = state_pool.tile([N, PPC, P], f16, name=f"g_{k}",
                                    tag=f"g_{k}")
                nc.vector.memset(t, 0.0)
                gst.append(t)
            t1st = [state_pool.tile([N, PPC, P], f32, name=f"t1_{k}",
                                    tag=f"t1_{k}") for k in range(2)]

            # software pipeline over supers ("ticks"): tick s runs the
            # state-independent front of super s (T/pcb/mask, brow on odd s)
            # and the delayed mid+tail of super s-1 (intra/pp, then
            # inter/stt/gmul which depend on the state recurrence).  The
            # one-super delay keeps the stt->gmul->stt recurrence off the
            # PE/DVE critical path.
            grp_of = {}
            for gi, (g0s, gs) in enumerate(GROUPS):
                for s in range(g0s, g0s + gs):
                    grp_of[s] = (gi, g0s, gs)
            tins = {}
            pts = {}
            msts = {}
            brows = {}
            ppts = {}
            pys = {}
            ysbs = {}
            tri_b = tri.unsqueeze(1).broadcast_to([CS, PPC, CS])

            def xin_of(s):
                gi, g0s, _ = grp_of[s]
                return tins[gi][:, s - g0s, :, 0:P]

            def ctin_of(s):
                gi, g0s, _ = grp_of[s]
                return tins[gi][:, s - g0s, :, P + N:LINE]

            for s in range(NSUP + 1):
                if s < NSUP:
                    gi, g0s, gs = grp_of[s]
                    if s == g0s:
                        tin = in_pool.tile([128, GS_MAX, PPC, LINE], f16,
                                           name="tin", tag="tin")
                        nc.sync.dma_start(out=tin[:, 0:gs],
                                          in_=Inp[:, g0s:g0s + gs])
                        tins[gi] = tin
                    # --- front(s): T + pcb + mask ---
                    btin = tins[gi][:, s - g0s, :, P:P + N]
                    ctin = ctin_of(s)
                    if s % 2 == 0:
                        pts[s // 2] = pt_pool.tile([CS, 2, PPC, N], f16,
                                                   name="pt", tag="pt")
                    pt = pts[s // 2]
                    for p in range(PPC):
                        nc.tensor.matmul(pt[:, s % 2, p, :], btin[:, p, :],
                                         ident, is_transpose=True)
                    pcb = pcb_pool.tile([CS, PPC, CS], f32, name="pcb",
                                        tag="pcb")
                    for p in range(PPC):
                        nc.tensor.matmul(pcb[:, p, :], btin[:, p, :],
                                         ctin[:, p, :], start=True, stop=True)
                    mst = mst_pool.tile([CS, PPC, CS], f16, name="mst",
                                        tag="mst")
                    nc.vector.tensor_mul(mst, pcb, tri_b)
                    msts[s] = mst
                    if s % 2 == 1:
                        brow = brow_pool.tile([CS, 2, PPC, N], f16,
                                              name="brow", tag="brow")
                        nc.scalar.activation(out=brow, in_=pts[s // 2],
                                             func=Act.Copy)
                        brows[s // 2] = brow

                if s >= 1:
                    # --- mid(s-1): intra + pp ---
                    d = s - 1
                    b = d // 2
                    xin = xin_of(d)
                    if d % 2 == 0:
                        pys[b] = py_pool.tile([CS, 2, PPC, P], f32,
                                              name="py", tag="py")
                    py = pys[b]
                    mst = msts.pop(d)
                    ppt = pp_pool.tile([N, PPC, P], f32, name="pp", tag="pp")
                    g0 = gst[d % 2]
                    g1 = gst[(d + 1) % 2]
                    ctin = ctin_of(d)
                    for p in range(PPC):
                        nc.tensor.matmul(py[:, d % 2, p, :], mst[:, p, :],
                                         xin[:, p, :], start=True,
                                         stop=(d == 0))
                        if d > 0:
                            nc.tensor.matmul(py[:, d % 2, p, :],
                                             ctin[:, p, :], g0[:, p, :],
                                             start=False, stop=True)
                        nc.tensor.matmul(ppt[:, p, :],
                                         brows[b][:, d % 2, p, :],
                                         xin[:, p, :], start=True, stop=True)

                    # --- tail(s-1): state advance ---
                    t1 = t1st[d % 2]
                    nc.vector.scalar_tensor_tensor(out=t1, in0=ppt,
                                                   scalar=1.0, in1=g0,
                                                   op0=Alu.mult, op1=Alu.add)
                    dnb = vecs[:, :, d:d + 1].broadcast_to([N, PPC, P])
                    nc.gpsimd.tensor_mul(g1, t1, dnb)

                    if d % 2 == 1:
                        # Y drain for the finished 2-super pair on ScalarE
                        w = b // 4
                        if b % 4 == 0:
                            ysbs[w] = yout_pool.tile([CS, 8, PPC, P], f16,
                                                     name="ysb", tag="ysb")
                        off = 2 * (b % 4)
                        nc.scalar.activation(out=ysbs[w][:, off:off + 2],
                                             in_=pys.pop(b), func=Act.Copy)
                        if b % 4 == 3:
                            nc.scalar.dma_start(out=Yp[:, 8 * w:8 * w + 8],
                                                in_=ysbs.pop(w))

    nc.compile()
    return nc


def _pack_inputs(X, A, Bm, Cm, ds):
    """Interleaved fp16 input image + decay vectors (mid-referenced)."""
    w = np.repeat(ds, CHUNK)                                     # [S]
    Acsg = np.cumsum(A.astype(np.float64) * w[None, :, None], axis=1)  # [B,S,H]

    Ac = Acsg.reshape(Bsz, NSUP, CS, H)
    a_end = Ac[:, :, -1, :]                                      # [B,NSUP,H]
    a_start = np.zeros_like(a_end)
    a_start[:, 1:] = a_end[:, :-1]
    r = 0.5 * (a_start + a_end)                                  # [B,NSUP,H]
    acs = Ac - r[:, :, None, :]                                  # centered, f64
    idf = np.exp(-acs).astype(np.float32)                        # [B,NSUP,CS,H]
    dfs = np.exp(acs).astype(np.float32)
    dnext = np.ones((Bsz, NSUP, H))
    dnext[:, :-1] = np.exp(r[:, 1:] - r[:, :-1])

    def pack_tmaj(T, D):   # [B,S,H,D] -> [NSUP, CS, pair, D]
        return T.reshape(Bsz, NSUP, CS, H, D).transpose(1, 2, 0, 3, 4) \
                .reshape(NSUP, CS, PAIRS, D)

    def pack_nmaj(T, D):   # [B,S,H,D] -> [NSUP, D, pair, CS]
        return T.reshape(Bsz, NSUP, CS, H, D).transpose(1, 4, 0, 3, 2) \
                .reshape(NSUP, D, PAIRS, CS)

    f16 = np.float16
    Xa = pack_tmaj(X, P)
    # free-axis folds: idf[j] for Bt, dfs[i] for Ct -> [NSUP, 1, pair, CS]
    idf_pair = idf.transpose(1, 0, 3, 2).reshape(NSUP, 1, PAIRS, CS)
    dfs_pair = dfs.transpose(1, 0, 3, 2).reshape(NSUP, 1, PAIRS, CS)
    Bta = pack_nmaj(Bm, N) * idf_pair
    Cta = pack_nmaj(Cm, N) * dfs_pair

    # interleave into [128, NSUP, PAIRS, LINE]
    Inq = np.concatenate([Xa.transpose(1, 0, 2, 3),
                          Bta.transpose(1, 0, 2, 3),
                          Cta.transpose(1, 0, 2, 3)], axis=-1).astype(f16)

    # dn per (pair, S), duplicated across partitions: [N, PAIRS, NSUP]
    dn = dnext.transpose(0, 2, 1).reshape(PAIRS, NSUP).astype(np.float32)
    vec = np.broadcast_to(dn[None, :, :], (N, PAIRS, NSUP))

    tri = (np.arange(CS)[None, :] >= np.arange(CS)[:, None]).astype(np.float32)
    idn = np.eye(N, dtype=f16)

    in_maps = []
    for k in range(NCORES):
        sl = slice(k * PPC, (k + 1) * PPC)
        in_maps.append({
            "Inp": np.ascontiguousarray(Inq[:, :, sl, :]),
            "Vec": np.ascontiguousarray(vec[:, sl, :]),
            "Tri": tri,
            "Idn": idn,
        })
    return in_maps


def kernel(X, A, Bm, Cm, log2_alpha_base, log2_beta, surprise_ema):
    X = np.ascontiguousarray(np.asarray(X, np.float32))
    A = np.ascontiguousarray(np.asarray(A, np.float32))
    Bm = np.ascontiguousarray(np.asarray(Bm, np.float32))
    Cm = np.ascontiguousarray(np.asarray(Cm, np.float32))
    log2_alpha_base = np.asarray(log2_alpha_base, np.float32)
    log2_beta = np.asarray(log2_beta, np.float32)
    surprise_ema = np.asarray(surprise_ema, np.float32)

    ds = host_gate_chain(X, A, Bm, log2_alpha_base, log2_beta, surprise_ema)
    in_maps = _pack_inputs(X, A, Bm, Cm, ds)

    if "nc" not in _CACHE:
        _CACHE["nc"] = build_nc()
    nc = _CACHE["nc"]

    from concourse.bass_utils import run_bass_kernel_spmd
    res = run_bass_kernel_spmd(nc, in_maps, core_ids=list(range(NCORES)))

    # gather: Yp [CS, NSUP, PPC, P] per core -> Y [B, S, H, P]
    Y = np.empty((PAIRS, NSUP, CS, P), np.float32)
    for k in range(NCORES):
        yk = res.results[k]["Yp"]                   # [CS, NSUP, PPC, P]
        Y[k * PPC:(k + 1) * PPC] = yk.transpose(2, 1, 0, 3)
    Y = Y.reshape(Bsz, H, S, P).transpose(0, 2, 1, 3)
    return np.ascontiguousarray(Y)
